# revision 2
# baseline (speedup 1.0000x reference)
"""LyraGemma3 sliding-window attention — Trainium2 Bass kernel, 8 NeuronCores.

Sharding: core = b*4 + h (b batch, h head-group). Each core owns vanilla head
h, lyra head 4+h, kv head h for batch b and produces output rows
[512h, 512h+512) of batch b. No collectives.

v2 vs baseline: all matmul operands bf16 (f32 psum accumulation), attention
operands SBUF-resident (no DRAM spill between phases), phase A (projections)
interleaved with phase C (attention) per 512-token group, k-rms-norm folded
into the Exp activation's per-partition scale (rope is linear so raw k works
for both streams), rstd computed with one Dsqrt activation, softmax
reciprocal via the fast DVE approximation, psum->sbuf copies moved to the
scalar/gpsimd engines, wo prefetched during attention.
"""

import sys

sys.path.insert(0, "/opt/trn_rl_repo")

import numpy as np
import ml_dtypes

import concourse.bass as bass
import concourse.tile as tile
from concourse import mybir
from concourse.tile import ScopedClock

F32 = mybir.dt.float32
F32R = mybir.dt.float32r
BF16 = mybir.dt.bfloat16
AF = mybir.ActivationFunctionType

B, S, HID = 2, 2048, 2560
H, KV, D = 8, 4, 256
WINDOW = 1024
THETA = 10000.0
EPS = 1e-6
SCALING = 256.0 ** (-0.5)  # 1/16

NKC = HID // 128  # 20 contraction chunks for projections
NST = 8           # s-tiles of 256 tokens
NT = S // 128     # 16 key tiles of 128
NQ = 4            # attention q-tiles of 512
MASK_NEG = -1e30


class SplitWaitTC(tile.TileContext):
    """This container's walrus encodes at most ONE semaphore wait per
    instruction; Tile emits multi-wait sync_info. Hoist extra waits onto
    preceding same-engine NOPs."""

    def _drain_and_barrier(self, tick_clock, wait_clock):
        nc = self.nc
        drain_inst = nc.sync.drain()
        wait_clock.add_sem_waits(
            drain_inst.ins, ScopedClock({None: tick_clock.global_clock})
        )
        self._split_multi_waits()
        nc.all_engine_barrier()
        popped = nc._tile_sem_poison_stack.pop()
        assert popped is self._sem_poison
        nc.clear_and_free_semaphores(list(self.sems.allocated().values()))
        nc.all_engine_barrier()

    def _split_multi_waits(self):
        nc = self.nc
        cur_bb = nc.cur_bb
        assert cur_bb is not None
        for f in nc.m.functions:
            for blk in f.blocks:
                insts = blk.instructions
                i = 0
                while i < len(insts):
                    inst = insts[i]
                    si = inst.sync_info
                    if si is not None and si.on_wait and len(si.on_wait) > 1:
                        waits = list(si.on_wait)
                        inst.sync_info = mybir.SyncInfo(
                            on_wait=waits[-1:], on_update=si.on_update
                        )
                        eng = inst.engine
                        for w in waits[:-1]:
                            nop = nc.engines[eng].nop()
                            nop.ins.sync_info = mybir.SyncInfo(
                                on_wait=[w], on_update=[]
                            )
                            cur_bb.bb.instructions.remove(nop.ins)
                            insts.insert(i, nop.ins)
                            i += 1
                    i += 1


def _mask_index(T, Q):
    """Mask tile for key-tile T against q-tile Q (queries [512Q,512Q+512)).
    Returns None (fully valid), 4+j (causal), or j'' (window edge)."""
    j = T - 4 * Q
    if j >= 0:
        return 4 + j
    if T >= 4 * Q - 4:
        return None
    return T - (4 * Q - 8)


def build_program():
    nc = bass.Bass()

    hsT = nc.declare_dram_parameter("hsT", [HID, S], BF16, isOutput=False)
    wq2 = nc.declare_dram_parameter("wq2", [HID, 512], BF16, isOutput=False)
    wk1 = nc.declare_dram_parameter("wk1", [HID, 256], BF16, isOutput=False)
    wv1 = nc.declare_dram_parameter("wv1", [HID, 256], BF16, isOutput=False)
    wo_d = nc.declare_dram_parameter("wo", [H * D, HID], BF16, isOutput=False)
    cos_d = nc.declare_dram_parameter("cos_t", [128, S], BF16, isOutput=False)
    sin_d = nc.declare_dram_parameter("sin_t", [128, S], BF16, isOutput=False)
    masks_d = nc.declare_dram_parameter("masks", [8 * 128, 512], BF16, isOutput=False)
    ident_d = nc.declare_dram_parameter("ident", [128, 128], BF16, isOutput=False)
    invq_d = nc.declare_dram_parameter("invq", [128, 2], BF16, isOutput=False)
    invk_d = nc.declare_dram_parameter("invk", [128, 256], BF16, isOutput=False)
    onec_d = nc.declare_dram_parameter("onec", [128, 1], BF16, isOutput=False)
    oner_d = nc.declare_dram_parameter("oner", [1, 128], F32R, isOutput=False)
    epsb_d = nc.declare_dram_parameter("epsb", [128, 2], F32, isOutput=False)
    out_d = nc.declare_dram_parameter("out", [512, HID], F32, isOutput=True)

    NWP = 4  # wq pieces (5 kc each); wk/wv in 2 pieces of 10 kc

    with SplitWaitTC(nc) as tc:
        with (
            tc.tile_pool(name="outer", bufs=1) as pO,
            tc.tile_pool(name="wo_pool", bufs=5) as pWo,
            tc.tile_pool(name="hst_pool", bufs=2) as pH,
        ):
            # ---- persistent SBUF tensors -------------------------------
            # priority-ordered input DMAs: first hst tile + first wq piece
            hst_t = {}

            def load_hst(st, split=1):
                t = pH.tile([128, NKC * 256], BF16, name="hst")
                hst_t[st] = t
                step = NKC // split
                for pi in range(split):
                    c0 = pi * step
                    nc.sync.dma_start(
                        t[:, c0 * 256 : (c0 + step) * 256].rearrange(
                            "p (c s) -> p c s", c=step
                        ),
                        hsT[:, st * 256 : (st + 1) * 256]
                        .rearrange("(c p) s -> c p s", p=128)
                        .transpose([1, 0, 2])[:, c0 : c0 + step, :],
                    )

            load_hst(0, split=2)
            wq_sb = pO.tile([128, NKC * 512], BF16, name="wq_sb")
            wq_pieces = [(0, 2), (2, 3), (5, 5), (10, 5), (15, 5)]
            for c0, w in wq_pieces:
                nc.sync.dma_start(
                    wq_sb[:, c0 * 512 : (c0 + w) * 512].rearrange(
                        "p (c d) -> p c d", c=w
                    ),
                    wq2[c0 * 128 : (c0 + w) * 128, :]
                    .rearrange("(c p) d -> c p d", p=128)
                    .transpose([1, 0, 2]),
                )
            wk_sb = pO.tile([128, NKC * 256], BF16, name="wk_sb")
            wv_sb = pO.tile([128, NKC * 256], BF16, name="wv_sb")
            for pc in range(2):
                nc.sync.dma_start(
                    wk_sb[:, pc * 10 * 256 : (pc + 1) * 10 * 256].rearrange(
                        "p (c d) -> p c d", c=10
                    ),
                    wk1[pc * 10 * 128 : (pc + 1) * 10 * 128, :]
                    .rearrange("(c p) d -> c p d", p=128)
                    .transpose([1, 0, 2]),
                )
                nc.sync.dma_start(
                    wv_sb[:, pc * 10 * 256 : (pc + 1) * 10 * 256].rearrange(
                        "p (c d) -> p c d", c=10
                    ),
                    wv1[pc * 10 * 128 : (pc + 1) * 10 * 128, :]
                    .rearrange("(c p) d -> c p d", p=128)
                    .transpose([1, 0, 2]),
                )
            invq = pO.tile([128, 2], BF16, name="invq")
            nc.sync.dma_start(invq[:], invq_d[:])
            invk = pO.tile([128, 256], BF16, name="invk")
            nc.sync.dma_start(invk[:], invk_d[:])
            onec = pO.tile([128, 1], BF16, name="onec")
            nc.sync.dma_start(onec[:], onec_d[:])
            oner = pO.tile([1, 128], F32R, name="oner")
            nc.sync.dma_start(oner[:], oner_d[:])
            epsb = pO.tile([128, 2], F32, name="epsb")
            nc.sync.dma_start(epsb[:], epsb_d[:])
            cos_sb = pO.tile([128, S], BF16, name="cos_sb")
            nc.sync.dma_start(cos_sb[:], cos_d[:])
            sin_sb = pO.tile([128, S], BF16, name="sin_sb")
            nc.sync.dma_start(sin_sb[:], sin_d[:])
            load_hst(1)
            ident = pO.tile([128, 128], BF16, name="ident")
            nc.sync.dma_start(ident[:], ident_d[:])
            masks_sb = pO.tile([128, 8 * 512], BF16, name="masks_sb")
            nc.sync.dma_start(
                masks_sb[:].rearrange("p (m s) -> p m s", m=8),
                masks_d[:].rearrange("(m p) s -> m p s", p=128).transpose([1, 0, 2]),
            )

            # persistent attention operands (written by A, read by C)
            qT = [pO.tile([128, S], BF16, name=f"qT{c}") for c in range(4)]
            kTr = [pO.tile([128, S], BF16, name=f"kTr{c}") for c in range(2)]
            kTn = [pO.tile([128, S], BF16, name=f"kTn{c}") for c in range(2)]
            vA = pO.tile([128, NT * 256], BF16, name="vA")
            krstdT = [
                pO.tile([128, 2], F32, name=f"krstdT{st}") for st in range(NST)
            ]
            outC = [
                [pO.tile([128, S], BF16, name=f"outC{s}{c}") for c in range(2)]
                for s in range(2)
            ]

            # wo prefetch tiles (DMAs issued early on SP; bufs gate reuse)
            wosb_t = {}
            co_blocks = [(cb * 512, 512) for cb in range(5)]

            for co, w in co_blocks:
                for kc in range(16):
                    t = pWo.tile([128, 512], BF16, name="wosb", bufs=8)
                    nc.sync.dma_start(
                        t[:], wo_d[kc * 128 : (kc + 1) * 128, co : co + w]
                    )
                    wosb_t[(co, kc)] = t

            with (
                tc.tile_pool(name="pA", bufs=2) as pA,
                tc.tile_pool(name="pAs", bufs=2) as pAs,
                tc.tile_pool(name="pC", bufs=4) as pC,
                tc.tile_pool(name="psA", bufs=2, space="PSUM") as psA,
                tc.tile_pool(name="psSm", bufs=1, space="PSUM") as psSm,
                tc.tile_pool(name="psS", bufs=3, space="PSUM") as psS,
                tc.tile_pool(name="psPo", bufs=1, space="PSUM") as psPo,
            ):

                def sm_tile():
                    # shared small-psum slot: [128,512] f32, 2 bufs rotating
                    return psSm.tile([128, 512], F32, name="sm")

                def phase_a(st):
                    s0 = st * 256
                    if st + 2 < NST:
                        load_hst(st + 2)
                    hst = hst_t[st]
                    # ---- q projection: 2 psum tiles of 2 d-chunks each --
                    qz = []
                    for half in range(2):
                        pq = psA.tile([128, 512], F32, name="pacc")
                        for g in range(2):
                            chunk = half * 2 + g
                            for kc in range(NKC):
                                nc.tensor.matmul(
                                    pq[:, g * 256 : (g + 1) * 256],
                                    wq_sb[
                                        :,
                                        kc * 512 + chunk * 128 : kc * 512
                                        + (chunk + 1) * 128,
                                    ],
                                    hst[:, kc * 256 : (kc + 1) * 256],
                                    start=(kc == 0),
                                    stop=(kc == NKC - 1),
                                    skip_group_check=True,
                                )
                        z = pA.tile([128, 512], BF16, name=f"qz{half}")
                        nc.scalar.activation(z[:], pq[:], AF.Copy)
                        qz.append(z)
                    # ---- k projection ----
                    pk = psA.tile([128, 512], F32, name="pacc")
                    for g in range(2):
                        for kc in range(NKC):
                            nc.tensor.matmul(
                                pk[:, g * 256 : (g + 1) * 256],
                                wk_sb[
                                    :, kc * 256 + g * 128 : kc * 256 + (g + 1) * 128
                                ],
                                hst[:, kc * 256 : (kc + 1) * 256],
                                start=(kc == 0),
                                stop=(kc == NKC - 1),
                                skip_group_check=True,
                            )
                    kz = pA.tile([128, 512], BF16, name="kz")
                    nc.scalar.activation(kz[:], pk[:], AF.Copy)
                    # ---- v projection (tokens on partitions) ----
                    pv = psA.tile([128, 512], F32, name="pacc")
                    for sm in range(2):
                        for kc in range(NKC):
                            nc.tensor.matmul(
                                pv[:, sm * 256 : (sm + 1) * 256],
                                hst[:, kc * 256 + sm * 128 : kc * 256 + sm * 128 + 128],
                                wv_sb[:, kc * 256 : (kc + 1) * 256],
                                start=(kc == 0),
                                stop=(kc == NKC - 1),
                                skip_group_check=True,
                            )
                    nc.scalar.activation(
                        vA[:, st * 512 : (st + 1) * 512], pv[:], AF.Copy
                    )

                    # ---- rms-norm stats ----
                    sq0 = pAs.tile([128, 512], BF16, name="sq0")
                    nc.gpsimd.tensor_mul(sq0[:], qz[0][:], qz[0][:])
                    sq1 = pAs.tile([128, 512], BF16, name="sq1")
                    nc.gpsimd.tensor_mul(sq1[:], qz[1][:], qz[1][:])
                    sqk = pAs.tile([128, 512], BF16, name="sqk")
                    nc.gpsimd.tensor_mul(sqk[:], kz[:], kz[:])
                    pnq = sm_tile()[0:1, :]
                    for h in range(2):
                        sq = sq0 if h == 0 else sq1
                        for c in range(2):
                            nc.tensor.matmul(
                                pnq[:, h * 256 : (h + 1) * 256],
                                invq[:, c : c + 1],
                                sq[:, c * 256 : (c + 1) * 256],
                                start=(c == 0),
                                stop=(c == 1),
                                skip_group_check=True,
                            )
                    pnkT = sm_tile()
                    for th in range(2):  # token half of this s-tile
                        for c in range(2):  # d-chunk
                            nc.tensor.matmul(
                                pnkT[:, th * 128 : th * 128 + 128],
                                sqk[:, c * 256 + th * 128 : c * 256 + th * 128 + 128],
                                invk[:, c * 128 : (c + 1) * 128],
                                start=(c == 0),
                                stop=(c == 1),
                                skip_group_check=True,
                            )
                    # rstd_q = 1/sqrt(pn/256+eps) = exp(-0.5*ln(pn/256+eps))
                    lnq = pAs.tile([1, 512], F32, name="lnq")
                    nc.scalar.activation(
                        lnq[:], pnq[:], AF.Ln, bias=epsb[0:1, 0:1], scale=1.0 / 256.0
                    )
                    rstdq = pAs.tile([1, 512], F32R, name="rstdq")
                    nc.scalar.activation(rstdq[:], lnq[:], AF.Exp, scale=-0.5)
                    # rstd_k*SCALING = 1/sqrt(pn+256*eps), already transposed
                    lnk = pAs.tile([128, 2], F32, name="lnk")
                    for th in range(2):
                        nc.scalar.activation(
                            lnk[:, th : th + 1],
                            pnkT[:, th * 128 : th * 128 + 1],
                            AF.Ln,
                            bias=epsb[:, 1:2],
                        )
                    nc.scalar.activation(krstdT[st][:], lnk[:], AF.Exp, scale=-0.5)
                    pbcq = sm_tile()
                    nc.tensor.matmul(pbcq[:], oner[:], rstdq[:], start=True, stop=True)
                    bcs = pAs.tile([128, 512], BF16, name="bcs")
                    nc.scalar.activation(bcs[:], pbcq[:], AF.Copy)


                    # ---- rope (bf16 on DVE) + raw-k copy ----
                    cs = cos_sb[:, s0 : s0 + 256]
                    sn = sin_sb[:, s0 : s0 + 256]

                    def rope2(z0, z1, bc, d0, d1):
                        t0 = pA.tile([128, 256], BF16, name="t0")
                        nc.vector.tensor_mul(t0[:], z0, cs)
                        t1 = pA.tile([128, 256], BF16, name="t1")
                        nc.vector.tensor_mul(t1[:], z1, sn)
                        u0 = pA.tile([128, 256], BF16, name="u0")
                        nc.vector.tensor_sub(u0[:], t0[:], t1[:])
                        t2 = pA.tile([128, 256], BF16, name="t2")
                        nc.vector.tensor_mul(t2[:], z1, cs)
                        t3 = pA.tile([128, 256], BF16, name="t3")
                        nc.vector.tensor_mul(t3[:], z0, sn)
                        if bc is not None:
                            u1 = pA.tile([128, 256], BF16, name="u1")
                            nc.vector.tensor_add(u1[:], t2[:], t3[:])
                            nc.vector.tensor_mul(d0, u0[:], bc)
                            nc.vector.tensor_mul(d1, u1[:], bc)
                        else:
                            nc.vector.tensor_copy(d0, u0[:])
                            nc.vector.tensor_add(d1, t2[:], t3[:])

                    for h in range(2):
                        rope2(
                            qz[h][:, 0:256],
                            qz[h][:, 256:512],
                            bcs[:, h * 256 : (h + 1) * 256],
                            qT[2 * h][:, s0 : s0 + 256],
                            qT[2 * h + 1][:, s0 : s0 + 256],
                        )
                    rope2(
                        kz[:, 0:256],
                        kz[:, 256:512],
                        None,
                        kTr[0][:, s0 : s0 + 256],
                        kTr[1][:, s0 : s0 + 256],
                    )
                    nc.gpsimd.tensor_copy(kTn[0][:, s0 : s0 + 256], kz[:, 0:256])
                    nc.gpsimd.tensor_copy(kTn[1][:, s0 : s0 + 256], kz[:, 256:512])

                def phase_c(Q, po_from_pacc=None):
                    for stream in range(2):
                        kT = kTr if stream == 0 else kTn
                        q0 = qT[2 * stream]
                        q1 = qT[2 * stream + 1]
                        T_lo = max(0, 4 * Q - 8)
                        T_hi = 4 * Q + 3
                        if po_from_pacc is not None and stream == po_from_pacc:
                            po0 = psA.tile([128, 512], F32, name="pacc")
                            po1 = psA.tile([128, 512], F32, name="pacc")
                        else:
                            po0 = psPo.tile([128, 512], F32, name="po0")
                            po1 = psPo.tile([128, 512], F32, name="po1")
                        psm = sm_tile()[0:1, :]
                        for ti, T in enumerate(range(T_lo, T_hi + 1)):
                            pss = psS.tile([128, 512], F32, name="pss")
                            nc.tensor.matmul(
                                pss[:],
                                kT[0][:, T * 128 : (T + 1) * 128],
                                q0[:, Q * 512 : (Q + 1) * 512],
                                start=True,
                                stop=False,
                            )
                            nc.tensor.matmul(
                                pss[:],
                                kT[1][:, T * 128 : (T + 1) * 128],
                                q1[:, Q * 512 : (Q + 1) * 512],
                                start=False,
                                stop=True,
                            )
                            midx = _mask_index(T, Q)
                            if midx is not None:
                                nc.vector.tensor_add(
                                    pss[:],
                                    pss[:],
                                    masks_sb[:, midx * 512 : (midx + 1) * 512],
                                )
                            probs = pC.tile([128, 512], BF16, name="probs")
                            nc.scalar.activation(
                                probs[:],
                                pss[:],
                                AF.Exp,
                                scale=krstdT[T // 2][:, T % 2 : T % 2 + 1],
                            )
                            first = T == T_lo
                            last = T == T_hi
                            nc.tensor.matmul(
                                psm[:], onec[:], probs[:], start=first, stop=last
                            )
                            nc.tensor.matmul(
                                po0[:],
                                vA[:, T * 256 : T * 256 + 128],
                                probs[:],
                                start=first,
                                stop=last,
                            )
                            nc.tensor.matmul(
                                po1[:],
                                vA[:, T * 256 + 128 : T * 256 + 256],
                                probs[:],
                                start=first,
                                stop=last,
                            )
                        lnm = pC.tile([1, 512], F32, name="lnm", bufs=2)
                        nc.scalar.activation(lnm[:], psm[:], AF.Ln)
                        rstCr = pC.tile([1, 512], F32R, name="rstCr", bufs=2)
                        nc.scalar.activation(rstCr[:], lnm[:], AF.Exp, scale=-1.0)
                        pbcC = sm_tile()
                        nc.tensor.matmul(
                            pbcC[:], oner[:], rstCr[:], start=True, stop=True
                        )
                        bcsC = pC.tile([128, 512], BF16, name="bcsC", bufs=2)
                        nc.scalar.activation(bcsC[:], pbcC[:], AF.Copy)
                        with nc.allow_low_precision(reason="attn out bf16"):
                            nc.vector.tensor_mul(
                                outC[stream][0][:, Q * 512 : (Q + 1) * 512],
                                po0[:],
                                bcsC[:],
                            )
                            nc.vector.tensor_mul(
                                outC[stream][1][:, Q * 512 : (Q + 1) * 512],
                                po1[:],
                                bcsC[:],
                            )

                # interleave: A(0),A(1),C(0),A(2),A(3),C(1),A(4)..A(7)
                for st in range(NST):
                    phase_a(st)
                    if st in (1, 3):
                        phase_c(st // 2)
                # post-A attention: second stream borrows pacc banks
                phase_c(2, po_from_pacc=1)
                phase_c(3, po_from_pacc=1)

            # ================= PHASE D: output projection ================
            with (
                tc.tile_pool(name="pD", bufs=3) as pD,
                tc.tile_pool(name="pDps", bufs=1, space="PSUM") as psD,
            ):
                for cb, (co, w) in enumerate(co_blocks):
                    pos = [
                        psD.tile([128, 512], F32, name=f"pD{m}{cb % 2}")
                        for m in range(4)
                    ]
                    for kc in range(16):
                        wosb = wosb_t[(co, kc)]
                        j, dc = kc // 2, kc % 2
                        for m in range(4):
                            stream, m0 = m // 2, (m % 2) * 128
                            lhsT = outC[stream][dc][:].rearrange(
                                "p (m j) -> p m j", j=8
                            )[:, m0 : m0 + 128, j : j + 1]
                            nc.tensor.matmul(
                                pos[m][:],
                                lhsT,
                                wosb[:],
                                start=(kc == 0),
                                stop=(kc == 15),
                            )
                    for m in range(4):
                        ost = pD.tile([128, 512], F32, name="ost")
                        if m % 2 == 0:
                            nc.scalar.activation(ost[:], pos[m][:], AF.Copy)
                        else:
                            nc.vector.tensor_copy(ost[:], pos[m][:])
                        nc.gpsimd.dma_start(
                            out_d[m * 128 : (m + 1) * 128, co : co + 512],
                            ost[:],
                        )
    return nc


def _host_inputs(hidden_states, wq, wk, wv, wo, q_norm_w, k_norm_w):
    """Build the 8 per-core input maps (all host-side numpy prep)."""
    bf16 = ml_dtypes.bfloat16
    hs = np.asarray(hidden_states, dtype=np.float32)
    wq = np.asarray(wq, dtype=np.float32)
    wk = np.asarray(wk, dtype=np.float32)
    wv = np.asarray(wv, dtype=np.float32)
    wo = np.ascontiguousarray(np.asarray(wo, dtype=np.float32).astype(bf16))
    qnw = np.asarray(q_norm_w, dtype=np.float32)
    knw = np.asarray(k_norm_w, dtype=np.float32)

    hsT = [np.ascontiguousarray(hs[b].T.astype(bf16)) for b in range(B)]

    inv_freq = 1.0 / (THETA ** (np.arange(0, D, 2, dtype=np.float32) / D))
    ang = np.outer(inv_freq, np.arange(S, dtype=np.float32))  # (128, S)
    cos_t = np.ascontiguousarray(np.cos(ang)).astype(bf16)
    sin_t = np.ascontiguousarray(np.sin(ang)).astype(bf16)

    x = np.arange(128)[:, None]
    y = np.arange(512)[None, :]
    masks = np.empty((8, 128, 512), np.float32)
    for jj in range(4):  # window-edge: valid iff y < x + 128*jj
        masks[jj] = np.where(y < x + 128 * jj, 0.0, MASK_NEG)
    for j in range(4):  # causal: valid iff y >= x + 128*j
        masks[4 + j] = np.where(y >= x + 128 * j, 0.0, MASK_NEG)
    masks = np.ascontiguousarray(masks.reshape(8 * 128, 512)).astype(bf16)

    invq = np.ascontiguousarray(
        ((1.0 + qnw) ** -2).reshape(2, 128).T.astype(bf16)
    )
    invk = np.ascontiguousarray(
        np.repeat(((1.0 + knw) ** -2).reshape(2, 128).T, 128, axis=1).astype(bf16)
    )
    onec = np.ones((128, 1), bf16)
    oner = np.ones((1, 128), np.float32)

    qs = 1.0 + qnw
    ks = 1.0 + knw
    in_maps = []
    for core in range(8):
        b, h = core // 4, core % 4
        wq2 = np.concatenate(
            [
                wq[:, h * D : (h + 1) * D] * qs[None, :],
                wq[:, (4 + h) * D : (5 + h) * D] * qs[None, :],
            ],
            axis=1,
        )
        in_maps.append(
            {
                "hsT": hsT[b],
                "wq2": np.ascontiguousarray(wq2.astype(bf16)),
                "wk1": np.ascontiguousarray(
                    (wk[:, h * D : (h + 1) * D] * ks[None, :]).astype(bf16)
                ),
                "wv1": np.ascontiguousarray(wv[:, h * D : (h + 1) * D].astype(bf16)),
                "wo": wo,
                "cos_t": cos_t,
                "sin_t": sin_t,
                "masks": masks,
                "ident": np.ascontiguousarray(np.eye(128).astype(bf16)),
                "invq": invq,
                "invk": invk,
                "onec": onec,
                "oner": oner,
                "epsb": np.tile(np.array([[EPS, 256.0 * EPS]], np.float32), (128, 1)),
            }
        )
    return in_maps


_PROGRAM = None


def kernel(hidden_states, wq, wk, wv, wo, q_norm_w, k_norm_w):
    global _PROGRAM
    from concourse.bass_utils import run_bass_kernel_spmd

    if _PROGRAM is None:
        _PROGRAM = build_program()
    in_maps = _host_inputs(hidden_states, wq, wk, wv, wo, q_norm_w, k_norm_w)
    res = run_bass_kernel_spmd(_PROGRAM, in_maps, core_ids=list(range(8)))
    out = np.empty((B, S, HID), np.float32)
    for core in range(8):
        b, h = core // 4, core % 4
        out[b, h * 512 : (h + 1) * 512, :] = res.results[core]["out"]
    return out


# revision 3
# speedup vs baseline: 1.0092x; 1.0092x over previous
"""LyraGemma3 sliding-window attention — Trainium2 Bass kernel, 8 NeuronCores.

Sharding: core = b*4 + h (b batch, h head-group). Each core owns vanilla head
h, lyra head 4+h, kv head h for batch b and produces output rows
[512h, 512h+512) of batch b. No collectives.

v2 vs baseline: all matmul operands bf16 (f32 psum accumulation), attention
operands SBUF-resident (no DRAM spill between phases), phase A (projections)
interleaved with phase C (attention) per 512-token group, k-rms-norm folded
into the Exp activation's per-partition scale (rope is linear so raw k works
for both streams), rstd computed with one Dsqrt activation, softmax
reciprocal via the fast DVE approximation, psum->sbuf copies moved to the
scalar/gpsimd engines, wo prefetched during attention.
"""

import sys

sys.path.insert(0, "/opt/trn_rl_repo")

import numpy as np
import ml_dtypes

import concourse.bass as bass
import concourse.tile as tile
from concourse import mybir
from concourse.tile import ScopedClock

F32 = mybir.dt.float32
F32R = mybir.dt.float32r
BF16 = mybir.dt.bfloat16
AF = mybir.ActivationFunctionType

B, S, HID = 2, 2048, 2560
H, KV, D = 8, 4, 256
WINDOW = 1024
THETA = 10000.0
EPS = 1e-6
SCALING = 256.0 ** (-0.5)  # 1/16

NKC = HID // 128  # 20 contraction chunks for projections
NST = 8           # s-tiles of 256 tokens
NT = S // 128     # 16 key tiles of 128
NQ = 4            # attention q-tiles of 512
MASK_NEG = -1e30


class SplitWaitTC(tile.TileContext):
    """This container's walrus encodes at most ONE semaphore wait per
    instruction; Tile emits multi-wait sync_info. Hoist extra waits onto
    preceding same-engine NOPs."""

    def _drain_and_barrier(self, tick_clock, wait_clock):
        nc = self.nc
        drain_inst = nc.sync.drain()
        wait_clock.add_sem_waits(
            drain_inst.ins, ScopedClock({None: tick_clock.global_clock})
        )
        self._split_multi_waits()
        nc.all_engine_barrier()
        popped = nc._tile_sem_poison_stack.pop()
        assert popped is self._sem_poison
        nc.clear_and_free_semaphores(list(self.sems.allocated().values()))
        nc.all_engine_barrier()

    def _split_multi_waits(self):
        nc = self.nc
        cur_bb = nc.cur_bb
        assert cur_bb is not None
        for f in nc.m.functions:
            for blk in f.blocks:
                insts = blk.instructions
                i = 0
                while i < len(insts):
                    inst = insts[i]
                    si = inst.sync_info
                    if si is not None and si.on_wait and len(si.on_wait) > 1:
                        waits = list(si.on_wait)
                        inst.sync_info = mybir.SyncInfo(
                            on_wait=waits[-1:], on_update=si.on_update
                        )
                        eng = inst.engine
                        for w in waits[:-1]:
                            nop = nc.engines[eng].nop()
                            nop.ins.sync_info = mybir.SyncInfo(
                                on_wait=[w], on_update=[]
                            )
                            cur_bb.bb.instructions.remove(nop.ins)
                            insts.insert(i, nop.ins)
                            i += 1
                    i += 1


def _mask_index(T, Q):
    """Mask tile for key-tile T against q-tile Q (queries [512Q,512Q+512)).
    Returns None (fully valid), 4+j (causal), or j'' (window edge)."""
    j = T - 4 * Q
    if j >= 0:
        return 4 + j
    if T >= 4 * Q - 4:
        return None
    return T - (4 * Q - 8)


def build_program():
    nc = bass.Bass()

    hsT = nc.declare_dram_parameter("hsT", [HID, S], BF16, isOutput=False)
    wq2 = nc.declare_dram_parameter("wq2", [HID, 512], BF16, isOutput=False)
    wk1 = nc.declare_dram_parameter("wk1", [HID, 256], BF16, isOutput=False)
    wv1 = nc.declare_dram_parameter("wv1", [HID, 256], BF16, isOutput=False)
    wo_d = nc.declare_dram_parameter("wo", [H * D, HID], BF16, isOutput=False)
    cos_d = nc.declare_dram_parameter("cos_t", [128, S], BF16, isOutput=False)
    sin_d = nc.declare_dram_parameter("sin_t", [128, S], BF16, isOutput=False)
    masks_d = nc.declare_dram_parameter("masks", [8 * 128, 512], BF16, isOutput=False)
    ident_d = nc.declare_dram_parameter("ident", [128, 128], BF16, isOutput=False)
    invq_d = nc.declare_dram_parameter("invq", [128, 2], BF16, isOutput=False)
    invk_d = nc.declare_dram_parameter("invk", [128, 256], BF16, isOutput=False)
    onec_d = nc.declare_dram_parameter("onec", [128, 1], BF16, isOutput=False)
    oner_d = nc.declare_dram_parameter("oner", [1, 128], F32R, isOutput=False)
    epsb_d = nc.declare_dram_parameter("epsb", [128, 2], F32, isOutput=False)
    out_d = nc.declare_dram_parameter("out", [512, HID], F32, isOutput=True)

    NWP = 4  # wq pieces (5 kc each); wk/wv in 2 pieces of 10 kc

    with SplitWaitTC(nc) as tc:
        with (
            tc.tile_pool(name="outer", bufs=1) as pO,
            tc.tile_pool(name="wo_pool", bufs=5) as pWo,
            tc.tile_pool(name="hst_pool", bufs=2) as pH,
        ):
            # ---- persistent SBUF tensors -------------------------------
            # priority-ordered input DMAs: first hst tile + first wq piece
            hst_t = {}

            def load_hst(st, split=1):
                t = pH.tile([128, NKC * 256], BF16, name="hst")
                hst_t[st] = t
                step = NKC // split
                for pi in range(split):
                    c0 = pi * step
                    nc.sync.dma_start(
                        t[:, c0 * 256 : (c0 + step) * 256].rearrange(
                            "p (c s) -> p c s", c=step
                        ),
                        hsT[:, st * 256 : (st + 1) * 256]
                        .rearrange("(c p) s -> c p s", p=128)
                        .transpose([1, 0, 2])[:, c0 : c0 + step, :],
                    )

            load_hst(0, split=2)
            wq_sb = pO.tile([128, NKC * 512], BF16, name="wq_sb")
            wq_pieces = [(0, 2), (2, 3), (5, 5), (10, 5), (15, 5)]
            for c0, w in wq_pieces:
                nc.sync.dma_start(
                    wq_sb[:, c0 * 512 : (c0 + w) * 512].rearrange(
                        "p (c d) -> p c d", c=w
                    ),
                    wq2[c0 * 128 : (c0 + w) * 128, :]
                    .rearrange("(c p) d -> c p d", p=128)
                    .transpose([1, 0, 2]),
                )
            wk_sb = pO.tile([128, NKC * 256], BF16, name="wk_sb")
            wv_sb = pO.tile([128, NKC * 256], BF16, name="wv_sb")
            for pc in range(2):
                nc.sync.dma_start(
                    wk_sb[:, pc * 10 * 256 : (pc + 1) * 10 * 256].rearrange(
                        "p (c d) -> p c d", c=10
                    ),
                    wk1[pc * 10 * 128 : (pc + 1) * 10 * 128, :]
                    .rearrange("(c p) d -> c p d", p=128)
                    .transpose([1, 0, 2]),
                )
                nc.sync.dma_start(
                    wv_sb[:, pc * 10 * 256 : (pc + 1) * 10 * 256].rearrange(
                        "p (c d) -> p c d", c=10
                    ),
                    wv1[pc * 10 * 128 : (pc + 1) * 10 * 128, :]
                    .rearrange("(c p) d -> c p d", p=128)
                    .transpose([1, 0, 2]),
                )
            invq = pO.tile([128, 2], BF16, name="invq")
            nc.sync.dma_start(invq[:], invq_d[:])
            invk = pO.tile([128, 256], BF16, name="invk")
            nc.sync.dma_start(invk[:], invk_d[:])
            onec = pO.tile([128, 1], BF16, name="onec")
            nc.sync.dma_start(onec[:], onec_d[:])
            oner = pO.tile([1, 128], F32R, name="oner")
            nc.sync.dma_start(oner[:], oner_d[:])
            epsb = pO.tile([128, 2], F32, name="epsb")
            nc.sync.dma_start(epsb[:], epsb_d[:])
            cos_sb = pO.tile([128, S], BF16, name="cos_sb")
            nc.sync.dma_start(cos_sb[:], cos_d[:])
            sin_sb = pO.tile([128, S], BF16, name="sin_sb")
            nc.sync.dma_start(sin_sb[:], sin_d[:])
            load_hst(1)
            ident = pO.tile([128, 128], BF16, name="ident")
            nc.sync.dma_start(ident[:], ident_d[:])
            masks_sb = pO.tile([128, 8 * 512], BF16, name="masks_sb")
            nc.sync.dma_start(
                masks_sb[:].rearrange("p (m s) -> p m s", m=8),
                masks_d[:].rearrange("(m p) s -> m p s", p=128).transpose([1, 0, 2]),
            )

            # persistent attention operands (written by A, read by C)
            qT = [pO.tile([128, S], BF16, name=f"qT{c}") for c in range(4)]
            kTr = [pO.tile([128, S], BF16, name=f"kTr{c}") for c in range(2)]
            kTn = [pO.tile([128, S], BF16, name=f"kTn{c}") for c in range(2)]
            vA = pO.tile([128, NT * 256], BF16, name="vA")
            krstdT = [
                pO.tile([128, 2], F32, name=f"krstdT{st}") for st in range(NST)
            ]
            outC = [
                [pO.tile([128, S], BF16, name=f"outC{s}{c}") for c in range(2)]
                for s in range(2)
            ]

            # wo prefetch tiles (DMAs issued early on SP; bufs gate reuse)
            wosb_t = {}
            co_blocks = [(cb * 512, 512) for cb in range(5)]

            for co, w in co_blocks:
                for kc in range(16):
                    t = pWo.tile([128, 512], BF16, name="wosb", bufs=8)
                    nc.sync.dma_start(
                        t[:], wo_d[kc * 128 : (kc + 1) * 128, co : co + w]
                    )
                    wosb_t[(co, kc)] = t

            with (
                tc.tile_pool(name="pA", bufs=2) as pA,
                tc.tile_pool(name="pAs", bufs=2) as pAs,
                tc.tile_pool(name="pC", bufs=4) as pC,
                tc.tile_pool(name="psA", bufs=2, space="PSUM") as psA,
                tc.tile_pool(name="psSm", bufs=1, space="PSUM") as psSm,
                tc.tile_pool(name="psS", bufs=3, space="PSUM") as psS,
                tc.tile_pool(name="psPo", bufs=1, space="PSUM") as psPo,
            ):

                def sm_tile():
                    # shared small-psum slot: [128,512] f32, 2 bufs rotating
                    return psSm.tile([128, 512], F32, name="sm")

                pending_fin = []

                def flush_fin():
                    while pending_fin:
                        pending_fin.pop(0)()

                def phase_a(st):
                    flush_fin()
                    s0 = st * 256
                    if st + 2 < NST:
                        load_hst(st + 2)
                    hst = hst_t[st]
                    # ---- q projection: 2 psum tiles of 2 d-chunks each --
                    qz = []
                    for half in range(2):
                        pq = psA.tile([128, 512], F32, name="pacc")
                        for g in range(2):
                            chunk = half * 2 + g
                            for kc in range(NKC):
                                nc.tensor.matmul(
                                    pq[:, g * 256 : (g + 1) * 256],
                                    wq_sb[
                                        :,
                                        kc * 512 + chunk * 128 : kc * 512
                                        + (chunk + 1) * 128,
                                    ],
                                    hst[:, kc * 256 : (kc + 1) * 256],
                                    start=(kc == 0),
                                    stop=(kc == NKC - 1),
                                    skip_group_check=True,
                                )
                        z = pA.tile([128, 512], BF16, name=f"qz{half}")
                        nc.scalar.activation(z[:], pq[:], AF.Copy)
                        qz.append(z)
                    sq0 = pAs.tile([128, 512], BF16, name="sq0")
                    nc.gpsimd.tensor_mul(sq0[:], qz[0][:], qz[0][:])
                    sq1 = pAs.tile([128, 512], BF16, name="sq1")
                    nc.gpsimd.tensor_mul(sq1[:], qz[1][:], qz[1][:])
                    # ---- k projection ----
                    pk = psA.tile([128, 512], F32, name="pacc")
                    for g in range(2):
                        for kc in range(NKC):
                            nc.tensor.matmul(
                                pk[:, g * 256 : (g + 1) * 256],
                                wk_sb[
                                    :, kc * 256 + g * 128 : kc * 256 + (g + 1) * 128
                                ],
                                hst[:, kc * 256 : (kc + 1) * 256],
                                start=(kc == 0),
                                stop=(kc == NKC - 1),
                                skip_group_check=True,
                            )
                    kz = pA.tile([128, 512], BF16, name="kz")
                    nc.scalar.activation(kz[:], pk[:], AF.Copy)
                    sqk = pAs.tile([128, 512], BF16, name="sqk")
                    nc.gpsimd.tensor_mul(sqk[:], kz[:], kz[:])
                    # q-stats emitted here: scalar Ln/Exp round-trip overlaps
                    # the v projection below, so pbcq never stalls the PE
                    pnq = sm_tile()[0:1, :]
                    for h in range(2):
                        sq = sq0 if h == 0 else sq1
                        for c in range(2):
                            nc.tensor.matmul(
                                pnq[:, h * 256 : (h + 1) * 256],
                                invq[:, c : c + 1],
                                sq[:, c * 256 : (c + 1) * 256],
                                start=(c == 0),
                                stop=(c == 1),
                                skip_group_check=True,
                            )
                    lnq = pAs.tile([1, 512], F32, name="lnq")
                    nc.scalar.activation(
                        lnq[:], pnq[:], AF.Ln, bias=epsb[0:1, 0:1], scale=1.0 / 256.0
                    )
                    rstdq = pAs.tile([1, 512], F32R, name="rstdq")
                    nc.scalar.activation(rstdq[:], lnq[:], AF.Exp, scale=-0.5)
                    # ---- v projection (tokens on partitions) ----
                    pv = psA.tile([128, 512], F32, name="pacc")
                    for sm in range(2):
                        for kc in range(NKC):
                            nc.tensor.matmul(
                                pv[:, sm * 256 : (sm + 1) * 256],
                                hst[:, kc * 256 + sm * 128 : kc * 256 + sm * 128 + 128],
                                wv_sb[:, kc * 256 : (kc + 1) * 256],
                                start=(kc == 0),
                                stop=(kc == NKC - 1),
                                skip_group_check=True,
                            )
                    nc.scalar.activation(
                        vA[:, st * 512 : (st + 1) * 512], pv[:], AF.Copy
                    )
                    pnkT = sm_tile()
                    for th in range(2):  # token half of this s-tile
                        for c in range(2):  # d-chunk
                            nc.tensor.matmul(
                                pnkT[:, th * 128 : th * 128 + 128],
                                sqk[:, c * 256 + th * 128 : c * 256 + th * 128 + 128],
                                invk[:, c * 128 : (c + 1) * 128],
                                start=(c == 0),
                                stop=(c == 1),
                                skip_group_check=True,
                            )
                    pbcq = sm_tile()
                    nc.tensor.matmul(pbcq[:], oner[:], rstdq[:], start=True, stop=True)
                    bcs = pAs.tile([128, 512], BF16, name="bcs")
                    nc.scalar.activation(bcs[:], pbcq[:], AF.Copy)
                    # rstd_k*SCALING = 1/sqrt(pn+256*eps), already transposed
                    lnk = pAs.tile([128, 2], F32, name="lnk")
                    for th in range(2):
                        nc.scalar.activation(
                            lnk[:, th : th + 1],
                            pnkT[:, th * 128 : th * 128 + 1],
                            AF.Ln,
                            bias=epsb[:, 1:2],
                        )
                    nc.scalar.activation(krstdT[st][:], lnk[:], AF.Exp, scale=-0.5)


                    # ---- rope (bf16 on DVE) + raw-k copy ----
                    cs = cos_sb[:, s0 : s0 + 256]
                    sn = sin_sb[:, s0 : s0 + 256]

                    def rope2(z0, z1, bc, d0, d1):
                        t0 = pA.tile([128, 256], BF16, name="t0")
                        nc.vector.tensor_mul(t0[:], z0, cs)
                        t1 = pA.tile([128, 256], BF16, name="t1")
                        nc.vector.tensor_mul(t1[:], z1, sn)
                        u0 = pA.tile([128, 256], BF16, name="u0")
                        nc.vector.tensor_sub(u0[:], t0[:], t1[:])
                        t2 = pA.tile([128, 256], BF16, name="t2")
                        nc.vector.tensor_mul(t2[:], z1, cs)
                        t3 = pA.tile([128, 256], BF16, name="t3")
                        nc.vector.tensor_mul(t3[:], z0, sn)
                        if bc is not None:
                            u1 = pA.tile([128, 256], BF16, name="u1")
                            nc.vector.tensor_add(u1[:], t2[:], t3[:])
                            nc.vector.tensor_mul(d0, u0[:], bc)
                            nc.vector.tensor_mul(d1, u1[:], bc)
                        else:
                            nc.vector.tensor_copy(d0, u0[:])
                            nc.vector.tensor_add(d1, t2[:], t3[:])

                    for h in range(2):
                        rope2(
                            qz[h][:, 0:256],
                            qz[h][:, 256:512],
                            bcs[:, h * 256 : (h + 1) * 256],
                            qT[2 * h][:, s0 : s0 + 256],
                            qT[2 * h + 1][:, s0 : s0 + 256],
                        )
                    rope2(
                        kz[:, 0:256],
                        kz[:, 256:512],
                        None,
                        kTr[0][:, s0 : s0 + 256],
                        kTr[1][:, s0 : s0 + 256],
                    )
                    nc.gpsimd.tensor_copy(kTn[0][:, s0 : s0 + 256], kz[:, 0:256])
                    nc.gpsimd.tensor_copy(kTn[1][:, s0 : s0 + 256], kz[:, 256:512])

                def phase_c(Q, po_from_pacc=None):
                    flush_fin()
                    for stream in range(2):
                        kT = kTr if stream == 0 else kTn
                        q0 = qT[2 * stream]
                        q1 = qT[2 * stream + 1]
                        T_lo = max(0, 4 * Q - 8)
                        T_hi = 4 * Q + 3
                        if po_from_pacc is not None and stream == po_from_pacc:
                            po0 = psA.tile([128, 512], F32, name="pacc")
                            po1 = psA.tile([128, 512], F32, name="pacc")
                        else:
                            po0 = psPo.tile([128, 512], F32, name="po0")
                            po1 = psPo.tile([128, 512], F32, name="po1")
                        psm = sm_tile()[0:1, :]
                        for ti, T in enumerate(range(T_lo, T_hi + 1)):
                            pss = psS.tile([128, 512], F32, name="pss")
                            nc.tensor.matmul(
                                pss[:],
                                kT[0][:, T * 128 : (T + 1) * 128],
                                q0[:, Q * 512 : (Q + 1) * 512],
                                start=True,
                                stop=False,
                            )
                            nc.tensor.matmul(
                                pss[:],
                                kT[1][:, T * 128 : (T + 1) * 128],
                                q1[:, Q * 512 : (Q + 1) * 512],
                                start=False,
                                stop=True,
                            )
                            midx = _mask_index(T, Q)
                            if midx is not None:
                                nc.vector.tensor_add(
                                    pss[:],
                                    pss[:],
                                    masks_sb[:, midx * 512 : (midx + 1) * 512],
                                )
                            probs = pC.tile([128, 512], BF16, name="probs")
                            nc.scalar.activation(
                                probs[:],
                                pss[:],
                                AF.Exp,
                                scale=krstdT[T // 2][:, T % 2 : T % 2 + 1],
                            )
                            first = T == T_lo
                            last = T == T_hi
                            nc.tensor.matmul(
                                psm[:], onec[:], probs[:], start=first, stop=last
                            )
                            nc.tensor.matmul(
                                po0[:],
                                vA[:, T * 256 : T * 256 + 128],
                                probs[:],
                                start=first,
                                stop=last,
                            )
                            nc.tensor.matmul(
                                po1[:],
                                vA[:, T * 256 + 128 : T * 256 + 256],
                                probs[:],
                                start=first,
                                stop=last,
                            )
                        lnm = pC.tile([1, 512], F32, name="lnm", bufs=2)
                        nc.scalar.activation(lnm[:], psm[:], AF.Ln)
                        rstCr = pC.tile([1, 512], F32R, name="rstCr", bufs=2)
                        nc.scalar.activation(rstCr[:], lnm[:], AF.Exp, scale=-1.0)

                        def finalize(stream=stream, rstCr=rstCr, po0=po0, po1=po1):
                            pbcC = sm_tile()
                            nc.tensor.matmul(
                                pbcC[:], oner[:], rstCr[:], start=True, stop=True
                            )
                            bcsC = pC.tile([128, 512], BF16, name="bcsC", bufs=2)
                            nc.scalar.activation(bcsC[:], pbcC[:], AF.Copy)
                            with nc.allow_low_precision(reason="attn out bf16"):
                                nc.vector.tensor_mul(
                                    outC[stream][0][:, Q * 512 : (Q + 1) * 512],
                                    po0[:],
                                    bcsC[:],
                                )
                                nc.vector.tensor_mul(
                                    outC[stream][1][:, Q * 512 : (Q + 1) * 512],
                                    po1[:],
                                    bcsC[:],
                                )

                        if stream == 1:
                            pending_fin.append(finalize)
                        else:
                            finalize()

                # interleave: A(0),A(1),C(0),A(2),A(3),C(1),A(4)..A(7)
                for st in range(NST):
                    phase_a(st)
                    if st in (1, 3):
                        phase_c(st // 2)
                # post-A attention: second stream borrows pacc banks
                phase_c(2, po_from_pacc=1)
                phase_c(3, po_from_pacc=1)
                flush_fin()

            # ================= PHASE D: output projection ================
            with (
                tc.tile_pool(name="pD", bufs=3) as pD,
                tc.tile_pool(name="pDps", bufs=1, space="PSUM") as psD,
            ):
                for cb, (co, w) in enumerate(co_blocks):
                    pos = [
                        psD.tile([128, 512], F32, name=f"pD{m}{cb % 2}")
                        for m in range(4)
                    ]
                    for kc in range(16):
                        wosb = wosb_t[(co, kc)]
                        j, dc = kc // 2, kc % 2
                        for m in range(4):
                            stream, m0 = m // 2, (m % 2) * 128
                            lhsT = outC[stream][dc][:].rearrange(
                                "p (m j) -> p m j", j=8
                            )[:, m0 : m0 + 128, j : j + 1]
                            nc.tensor.matmul(
                                pos[m][:],
                                lhsT,
                                wosb[:],
                                start=(kc == 0),
                                stop=(kc == 15),
                            )
                    for m in range(4):
                        ost = pD.tile([128, 512], F32, name="ost")
                        if m % 2 == 0:
                            nc.scalar.activation(ost[:], pos[m][:], AF.Copy)
                        else:
                            nc.vector.tensor_copy(ost[:], pos[m][:])
                        nc.gpsimd.dma_start(
                            out_d[m * 128 : (m + 1) * 128, co : co + 512],
                            ost[:],
                        )
    return nc


def _host_inputs(hidden_states, wq, wk, wv, wo, q_norm_w, k_norm_w):
    """Build the 8 per-core input maps (all host-side numpy prep)."""
    bf16 = ml_dtypes.bfloat16
    hs = np.asarray(hidden_states, dtype=np.float32)
    wq = np.asarray(wq, dtype=np.float32)
    wk = np.asarray(wk, dtype=np.float32)
    wv = np.asarray(wv, dtype=np.float32)
    wo = np.ascontiguousarray(np.asarray(wo, dtype=np.float32).astype(bf16))
    qnw = np.asarray(q_norm_w, dtype=np.float32)
    knw = np.asarray(k_norm_w, dtype=np.float32)

    hsT = [np.ascontiguousarray(hs[b].T.astype(bf16)) for b in range(B)]

    inv_freq = 1.0 / (THETA ** (np.arange(0, D, 2, dtype=np.float32) / D))
    ang = np.outer(inv_freq, np.arange(S, dtype=np.float32))  # (128, S)
    cos_t = np.ascontiguousarray(np.cos(ang)).astype(bf16)
    sin_t = np.ascontiguousarray(np.sin(ang)).astype(bf16)

    x = np.arange(128)[:, None]
    y = np.arange(512)[None, :]
    masks = np.empty((8, 128, 512), np.float32)
    for jj in range(4):  # window-edge: valid iff y < x + 128*jj
        masks[jj] = np.where(y < x + 128 * jj, 0.0, MASK_NEG)
    for j in range(4):  # causal: valid iff y >= x + 128*j
        masks[4 + j] = np.where(y >= x + 128 * j, 0.0, MASK_NEG)
    masks = np.ascontiguousarray(masks.reshape(8 * 128, 512)).astype(bf16)

    invq = np.ascontiguousarray(
        ((1.0 + qnw) ** -2).reshape(2, 128).T.astype(bf16)
    )
    invk = np.ascontiguousarray(
        np.repeat(((1.0 + knw) ** -2).reshape(2, 128).T, 128, axis=1).astype(bf16)
    )
    onec = np.ones((128, 1), bf16)
    oner = np.ones((1, 128), np.float32)

    qs = 1.0 + qnw
    ks = 1.0 + knw
    in_maps = []
    for core in range(8):
        b, h = core // 4, core % 4
        wq2 = np.concatenate(
            [
                wq[:, h * D : (h + 1) * D] * qs[None, :],
                wq[:, (4 + h) * D : (5 + h) * D] * qs[None, :],
            ],
            axis=1,
        )
        in_maps.append(
            {
                "hsT": hsT[b],
                "wq2": np.ascontiguousarray(wq2.astype(bf16)),
                "wk1": np.ascontiguousarray(
                    (wk[:, h * D : (h + 1) * D] * ks[None, :]).astype(bf16)
                ),
                "wv1": np.ascontiguousarray(wv[:, h * D : (h + 1) * D].astype(bf16)),
                "wo": wo,
                "cos_t": cos_t,
                "sin_t": sin_t,
                "masks": masks,
                "ident": np.ascontiguousarray(np.eye(128).astype(bf16)),
                "invq": invq,
                "invk": invk,
                "onec": onec,
                "oner": oner,
                "epsb": np.tile(np.array([[EPS, 256.0 * EPS]], np.float32), (128, 1)),
            }
        )
    return in_maps


_PROGRAM = None


def kernel(hidden_states, wq, wk, wv, wo, q_norm_w, k_norm_w):
    global _PROGRAM
    from concourse.bass_utils import run_bass_kernel_spmd

    if _PROGRAM is None:
        _PROGRAM = build_program()
    in_maps = _host_inputs(hidden_states, wq, wk, wv, wo, q_norm_w, k_norm_w)
    res = run_bass_kernel_spmd(_PROGRAM, in_maps, core_ids=list(range(8)))
    out = np.empty((B, S, HID), np.float32)
    for core in range(8):
        b, h = core // 4, core % 4
        out[b, h * 512 : (h + 1) * 512, :] = res.results[core]["out"]
    return out
